# revision 23
# baseline (speedup 1.0000x reference)
"""De-stationary attention on 8 Trainium2 NeuronCores.

Problem: y = softmax((x Wq^T + bq)(x Wk^T + bk)^T * scale / (tau*x_std)) (x Wv^T + bv) Wo^T + bo
Shapes: x [4, 2048, 1024], 16 heads of 64 dims, tau=1, delta=0.

Sharding: core c handles batch b = c//2, head group g = c%2 (8 heads).
The de-stationary scale s = SCALE/x_std[b] is folded into Wq on the host.
Each core computes a partial y over its 8 heads; host sums the two
head-group partials per batch and adds bo + bv @ Wo.T (value bias passes
through softmax rows which sum to 1; the q.bk logit term is constant
along keys and cancels in softmax, so bk needs no device handling).

Device math per core (matmuls in float32r = TF32-like, 1 col/cyc):
  qT[j] [128, 2048] = (s*Wq_pair_j) x^T + s*bq   (pair j = heads 2j, 2j+1:
       even head dims on partitions 0:64, odd head on 64:128)
  kT[j] [128, 2048] = Wk_pair_j x^T               (same pair layout)
  v  [2048, 8*65]: per head 64 value cols + ones col (gives the softmax
       denominator l as row 64 of the PV output)
  attention per (pair j, tq half of 1024), per tk tile of 128 keys:
     S^T[tk, 1024] per head = kT_h-slice^T . qT_h   (K=64 matmuls in
       opposite PE row groups -> the two heads run concurrently)
     P = exp(S^T) over [128, 2048] (both heads in one PSUM tile):
       10/16 tk tiles on ACT (exact exp), 6/16 on DVE via the
       Schraudolph bit trick exp(x) ~= bits(int32(A*x + B)) (~3% max
       elementwise, ~9e-3 end-to-end vs the 2e-2 gate)
     psum_O[65, 2048] += v_aug[tk]^T . P  (row 64 accumulates l)
  normalize: O_norm = O * (1/l) bcast; staging copies on ACT+DVE,
     reciprocal on DVE, partition-broadcast via DMA, multiply on GPSIMD
  y_part[2048, 1024] = sum_pairs O_pair^T . wo_pair  (K=128)
"""

import os
import sys

for _p in ("/opt/trn_rl_repo", "/root/.axon_site/_ro/trn_rl_repo"):
    if os.path.isdir(_p) and _p not in sys.path:
        sys.path.insert(0, _p)

import numpy as np

import concourse.bass as bass
import concourse.mybir as mybir
import concourse.tile as tile
from concourse import bacc
from concourse.bass_utils import run_bass_kernel_spmd

F32 = mybir.dt.float32
F32R = mybir.dt.float32r
BF16 = mybir.dt.bfloat16
I16 = mybir.dt.int16
I32 = mybir.dt.int32
AF = mybir.ActivationFunctionType

B, T, D, H = 4, 2048, 1024, 16
HD = D // H          # 64
SCALE = HD ** -0.5
HG = H // 2          # 8 heads per core
EG = HG * HD         # 512 projection dims per core
N_CORES = 8

TQH = T // 4         # 512: tq quarter processed per attention inner loop
NQH = T // TQH       # 4 query chunks
NTK = T // 128       # 16 tk tiles
NPAIR = HG // 2      # 4 head pairs per core

# Row-group (K=64, 2-heads-concurrent) S matmuls trigger the PE HAM clock
# governor (such matmuls do not register as PE activity -> 1.2 GHz), so the
# default is zero-padded K=128 S matmuls at full clock.
ROWGRP = os.environ.get("KROWGRP", "0") == "1"
# tk tiles whose exp runs on DVE (Schraudolph); rest on ACT (exact).
DVE_TKS = frozenset(
    int(t) for t in os.environ.get("KDVE_TKS", "1,4,6,9,11,14").split(",") if t != ""
)
# bf16 Schraudolph: exp(x) ~= bits_as_bf16(int16(A16*x + B16))
SCH_A16 = float(np.float32(128.0 / np.log(2.0)))
SCH_B16 = float(np.float32(127 * 128 - 366393 / 65536.0))


def _build():
    nc = bacc.Bacc("TRN2", target_bir_lowering=False, debug=False)

    xt_d = nc.dram_tensor("xt", [D, T], F32, kind="ExternalInput")
    wq_d = nc.dram_tensor("wq", [D, EG], F32, kind="ExternalInput")
    wk_d = nc.dram_tensor("wk", [D, EG], F32, kind="ExternalInput")
    wv_d = nc.dram_tensor("wv", [D, EG], F32, kind="ExternalInput")
    wo_d = nc.dram_tensor("wo", [EG, D], F32, kind="ExternalInput")
    bq_d = nc.dram_tensor("bq", [EG], F32, kind="ExternalInput")
    if not ROWGRP:
        # mask[:, 0] = rows 0:64 one / rows 64:128 zero; mask[:, 1] inverted.
        mk_d = nc.dram_tensor("mask", [128, 2], F32, kind="ExternalInput")
    y_d = nc.dram_tensor("y", [T, D], F32, kind="ExternalOutput")
    # DRAM bounces for the l reshape (DVE reciprocal is ~8 cyc/elem, so run
    # it on a [32, 32] reshape where FD=32) and the rinv partition-broadcast
    # (SBUF DMA sources cannot have partition stride 0); distinct regions
    # per (j, tq) iteration
    ls_d = nc.dram_tensor("l_scratch", [NPAIR, NQH, 2 * TQH], F32)
    rs_d = nc.dram_tensor("rinv_scratch", [NPAIR, NQH, 2, TQH], F32)

    with tile.TileContext(nc) as tc:
        from contextlib import ExitStack
        with ExitStack() as octx:
            # ---- persistent pools (live through phases A-C) ----
            qk_pool = octx.enter_context(tc.tile_pool(name="qk", bufs=1))
            v_pool = octx.enter_context(tc.tile_pool(name="vp", bufs=1))

            qT = [qk_pool.tile([128, T], F32R, name=f"qT{j}", tag=f"qT{j}")
                  for j in range(NPAIR)]
            if ROWGRP:
                # pair layout: head 2j dims on partitions 0:64, head 2j+1
                # on 64:128. S matmuls use K=64 in opposite PE row groups.
                kT = [qk_pool.tile([128, T], F32R, name=f"kT{j}", tag=f"kT{j}")
                      for j in range(NPAIR)]
            else:
                # zero-padded per-head tiles (K=128 matmuls at full clock)
                kT = [qk_pool.tile([128, T], F32R, name=f"kT{h}", tag=f"kT{h}")
                      for h in range(HG)]
            v_sb = [v_pool.tile([128, HG * (HD + 1)], BF16, name=f"v{t}",
                                tag=f"v{t}") for t in range(NTK)]

            # ---- warm-up: trip the PE HAM to full clock while DMAs land
            with tc.tile_pool(name="wu", bufs=1) as wup, \
                 tc.tile_pool(name="wu_ps", bufs=1, space="PSUM") as wups:
                wu_t = wup.tile([128, 512], F32R, name="wu")
                nc.vector.memset(wu_t[:].bitcast(F32), 0.0)
                wu_ps = wups.tile([128, 512], F32, name="wups")
                for i in range(28):
                    nc.tensor.matmul(wu_ps[:], wu_t[:, 0:128], wu_t[:],
                                     start=True, stop=True,
                                     skip_group_check=True)

            # ---- phase A: projections ----
            with tc.tile_pool(name="pa", bufs=1) as pa, \
                 tc.tile_pool(name="pa_x", bufs=1) as pax, \
                 tc.tile_pool(name="pa_ps", bufs=4, space="PSUM") as paps:
                wq_t = [pa.tile([128, EG], F32R, name=f"wq{k}", tag=f"wq{k}")
                        for k in range(8)]
                wk_t = [pa.tile([128, EG], F32R, name=f"wk{k}", tag=f"wk{k}")
                        for k in range(8)]
                wv_t = [pa.tile([128, EG], F32R, name=f"wv{k}", tag=f"wv{k}")
                        for k in range(8)]
                bq_t = [pa.tile([128, 1], F32, name=f"bq{e}", tag=f"bq{e}")
                        for e in range(4)]
                # wq + first x tiles are needed first: split across both
                # HWDGE rings so phase A can start sooner; wk/wv afterwards.
                for k in range(8):
                    r = slice(k * 128, (k + 1) * 128)
                    eng = nc.sync if k < 4 else nc.scalar
                    eng.dma_start(wq_t[k][:], wq_d.ap()[r, :].bitcast(F32R))
                for e in range(4):
                    nc.sync.dma_start(bq_t[e][:],
                                      bq_d.ap()[e * 128:(e + 1) * 128])
                if not ROWGRP:
                    mk_t = pa.tile([128, 2], F32, name="mk")
                    nc.sync.dma_start(mk_t[:], mk_d.ap())
                for k in range(8):
                    r = slice(k * 128, (k + 1) * 128)
                    nc.sync.dma_start(wk_t[k][:], wk_d.ap()[r, :].bitcast(F32R))
                for k in range(8):
                    r = slice(k * 128, (k + 1) * 128)
                    nc.scalar.dma_start(wv_t[k][:], wv_d.ap()[r, :].bitcast(F32R))

                for tq in range(4):          # t quarters of 512
                    tsl = slice(tq * 512, (tq + 1) * 512)
                    x_t = [pax.tile([128, 512], F32R, name=f"x{k}", tag=f"x{k}",
                                    bufs=2)
                           for k in range(8)]
                    for k in range(8):
                        eng = nc.scalar if (tq == 0 and k < 4) else nc.sync
                        eng.dma_start(
                            x_t[k][:],
                            xt_d.ap()[k * 128:(k + 1) * 128, tsl].bitcast(F32R))
                    # q projection: out [e-tile 128, t 512], bias bq (ACT)
                    for e in range(4):
                        ps = paps.tile([128, 512], F32, name="pj", tag="pj")
                        esl = slice(e * 128, (e + 1) * 128)
                        for k in range(8):
                            nc.tensor.matmul(ps[:], wq_t[k][:, esl], x_t[k][:],
                                             start=(k == 0), stop=(k == 7))
                        nc.scalar.activation(qT[e][:, tsl], ps[:], AF.Identity,
                                             bias=bq_t[e][:, 0:1])
                    # k projection
                    for e in range(4):
                        ps = paps.tile([128, 512], F32, name="pj", tag="pj")
                        esl = slice(e * 128, (e + 1) * 128)
                        for k in range(8):
                            nc.tensor.matmul(ps[:], wk_t[k][:, esl], x_t[k][:],
                                             start=(k == 0), stop=(k == 7))
                        if ROWGRP:
                            nc.scalar.copy(kT[e][:, tsl], ps[:])
                        else:
                            for p in range(2):
                                nc.vector.tensor_scalar(
                                    kT[2 * e + p][:, tsl], ps[:],
                                    mk_t[:, p:p + 1], None,
                                    mybir.AluOpType.mult)
                    # v projection: out [tk-tile 128, e 512] -> 65-strided
                    for tt in range(4):
                        ps = paps.tile([128, 512], F32, name="pj", tag="pj")
                        ti = tq * 4 + tt
                        for k in range(8):
                            nc.tensor.matmul(
                                ps[:], x_t[k][:, tt * 128:(tt + 1) * 128],
                                wv_t[k][:], start=(k == 0), stop=(k == 7))
                        vre = v_sb[ti][:].rearrange("p (h c) -> p h c", c=HD + 1)
                        nc.vector.tensor_copy(
                            vre[:, :, 0:HD],
                            ps[:].rearrange("p (h c) -> p h c", c=HD))
                        nc.scalar.activation(vre[:, :, HD:HD + 1],
                                             ps[:].rearrange(
                                                 "p (h c) -> p h c", c=HD)[:, :, 0:1],
                                             AF.Identity, bias=1.0, scale=0.0)

            # ---- phases B+C pools ----
            with tc.tile_pool(name="pb", bufs=1) as pb:
                wo_t = [pb.tile([128, D], F32R, name=f"wo{j}", tag=f"wo{j}")
                        for j in range(NPAIR)]
                for j in range(NPAIR):
                    nc.sync.dma_start(
                        wo_t[j][:],
                        wo_d.ap()[j * 128:(j + 1) * 128, :].bitcast(F32R))
                o_sb = [pb.tile([128, T], F32R, name=f"o{j}", tag=f"o{j}")
                        for j in range(NPAIR)]

                # ---- phase B: attention ----
                # per (pair j, query chunk of 512): ps_s [128, 1024] holds
                # S^T for both heads (head p cols p*512), double-buffered so
                # S(tk+1) never waits on exp(tk); ps_o [65, 1024] accumulates
                # O + l over all 16 tk tiles. PV matmuls are emitted 2 tk
                # behind S so the in-order PE queue never stalls on exp.
                pbps_ctx = tc.tile_pool(name="pb_ps", bufs=1, space="PSUM")
                pbps = pbps_ctx.__enter__()
                for j in range(NPAIR):
                    for tq in range(NQH):
                        qsl = slice(tq * TQH, (tq + 1) * TQH)
                        ps_o = pbps.tile([65, 2 * TQH], F32, name="ps_o",
                                         tag="ps_o", bufs=2)
                        pts = [None] * NTK

                        def emit_s_exp(tk):
                            ksl = slice(tk * 128, (tk + 1) * 128)
                            ps_s = pbps.tile([128, 2 * TQH], F32, name="ps_s",
                                             tag="ps_s", bufs=2)
                            for p in range(2):
                                osl = slice(p * TQH, (p + 1) * TQH)
                                if ROWGRP:
                                    psl = slice(p * 64, (p + 1) * 64)
                                    nc.tensor.matmul(
                                        ps_s[:, osl], kT[j][psl, ksl],
                                        qT[j][psl, qsl],
                                        start=True, stop=True)
                                else:
                                    nc.tensor.matmul(
                                        ps_s[:, osl], kT[2 * j + p][:, ksl],
                                        qT[j][:, qsl],
                                        start=True, stop=True)
                            pt = pb.tile([128, 2 * TQH], BF16, name="pt",
                                         tag="pt", bufs=4)
                            if tk in DVE_TKS:
                                # Schraudolph exp on DVE: the int16(A*x + B)
                                # bit pattern read as bf16 ~= e^x
                                nc.vector.tensor_scalar(
                                    pt[:].bitcast(I16), ps_s[:],
                                    SCH_A16, SCH_B16,
                                    mybir.AluOpType.mult,
                                    mybir.AluOpType.add)
                            else:
                                nc.scalar.activation(pt[:], ps_s[:], AF.Exp)
                            pts[tk] = pt

                        def emit_pv(tk):
                            pt = pts[tk]
                            for p in range(2):
                                h = 2 * j + p
                                vcol = slice(h * (HD + 1), (h + 1) * (HD + 1))
                                osl = slice(p * TQH, (p + 1) * TQH)
                                nc.tensor.matmul(
                                    ps_o[0:65, osl], v_sb[tk][:, vcol],
                                    pt[:, osl],
                                    start=(tk == 0), stop=(tk == NTK - 1))
                            pts[tk] = None

                        for tk in range(NTK):
                            emit_s_exp(tk)
                            if tk >= 2:
                                emit_pv(tk - 2)
                        emit_pv(NTK - 2)
                        emit_pv(NTK - 1)

                        # normalize: O = psum_O[0:64 per head] * (1/l).
                        # staging tile [128, 512]: O_p0 rows 0:64 (ACT copy),
                        # O_p1 rows 64:128 (DVE copy, partition-shifted);
                        # l rows -> lr2 [2, 512] (DVE), recip (DVE),
                        # partition-broadcast (DMA bounce), multiply (GPSIMD).
                        stg = pb.tile([128, TQH], F32, name="stg", tag="stg",
                                      bufs=2)
                        lrw = pb.tile([33, TQH], F32, name="lrw", tag="lrw",
                                      bufs=2)
                        lrs = pb.tile([32, 32], F32, name="lrs", tag="lrs",
                                      bufs=2)
                        rrs = pb.tile([32, 32], F32, name="rrs", tag="rrs",
                                      bufs=2)
                        rbc = pb.tile([128, TQH], F32, name="rbc", tag="rbc",
                                      bufs=2)
                        nc.scalar.copy(stg[0:64, :], ps_o[0:64, 0:TQH])
                        nc.vector.tensor_copy(stg[64:128, :],
                                              ps_o[0:64, TQH:2 * TQH])
                        nc.vector.tensor_copy(lrw[0:1, :], ps_o[64:65, 0:TQH])
                        nc.scalar.copy(lrw[32:33, :],
                                       ps_o[64:65, TQH:2 * TQH])
                        # DVE reciprocal is ~8 cyc/elem: bounce l through DRAM
                        # into a [32, 32] layout so FD is only 32
                        nc.sync.dma_start(ls_d.ap()[j, tq, 0:TQH], lrw[0:1, :])
                        nc.sync.dma_start(ls_d.ap()[j, tq, TQH:2 * TQH],
                                          lrw[32:33, :])
                        nc.sync.dma_start(
                            lrs[:],
                            ls_d.ap()[j, tq].rearrange("(a b) -> a b", b=32))
                        nc.vector.reciprocal(rrs[:], lrs[:])
                        nc.sync.dma_start(
                            rs_d.ap()[j, tq].rearrange("a (c b) -> (a c) b",
                                                       b=32), rrs[:])
                        for p in range(2):
                            nc.sync.dma_start(
                                rbc[p * 64:(p + 1) * 64, :],
                                rs_d.ap()[j, tq, p, :][None, :]
                                .broadcast_to((64, TQH)))
                        nc.gpsimd.tensor_tensor(
                            o_sb[j][:, qsl], stg[:], rbc[:],
                            mybir.AluOpType.mult)

                pbps_ctx.__exit__(None, None, None)

                # ---- phase C: output projection ----
                pcps_ctx = tc.tile_pool(name="pc_ps", bufs=3, space="PSUM")
                pcps = pcps_ctx.__enter__()
                for tt in range(NTK):
                    tsl = slice(tt * 128, (tt + 1) * 128)
                    ps_y = pcps.tile([128, 1024], F32, name="py", tag="py")
                    for nk in range(2):
                        nsl = slice(nk * 512, (nk + 1) * 512)
                        for j in range(NPAIR):
                            nc.tensor.matmul(ps_y[:, nsl], o_sb[j][:, tsl],
                                             wo_t[j][:, nsl],
                                             start=(j == 0), stop=(j == NPAIR - 1))
                    y_t = pb.tile([128, 1024], F32, name="yt", tag="yt", bufs=2)
                    if tt % 2 == 0:
                        nc.scalar.copy(y_t[:], ps_y[:])
                    else:
                        nc.vector.tensor_copy(y_t[:], ps_y[:])
                    # all y stores on the scalar ring: the sync ring still
                    # drains the last attention-normalize chains
                    nc.scalar.dma_start(y_d.ap()[tsl, :], y_t[:])
                pcps_ctx.__exit__(None, None, None)

    nc.compile()
    return nc


_NC = None
_last_in_maps = None


def kernel(x, x_mean, x_std, Wq, bq, Wk, bk, Wv, bv, Wo, bo):
    global _NC
    if _NC is None:
        _NC = _build()

    x = np.asarray(x, dtype=np.float32)
    x_std = np.asarray(x_std, dtype=np.float32)
    Wq = np.asarray(Wq, dtype=np.float32)
    Wk = np.asarray(Wk, dtype=np.float32)
    Wv = np.asarray(Wv, dtype=np.float32)
    Wo = np.asarray(Wo, dtype=np.float32)
    bq = np.asarray(bq, dtype=np.float32)
    bv = np.asarray(bv, dtype=np.float32)
    bo = np.asarray(bo, dtype=np.float32)

    mask = np.zeros((128, 2), dtype=np.float32)
    mask[0:64, 0] = 1.0
    mask[64:128, 1] = 1.0
    in_maps = []
    for c in range(N_CORES):
        b, g = c // 2, c % 2
        s = np.float32(SCALE / float(x_std[b, 0, 0]))
        rows = slice(g * EG, (g + 1) * EG)
        im = {
            "xt": np.ascontiguousarray(x[b].T),
            "wq": np.ascontiguousarray((Wq[rows, :] * s).T),
            "wk": np.ascontiguousarray(Wk[rows, :].T),
            "wv": np.ascontiguousarray(Wv[rows, :].T),
            "wo": np.ascontiguousarray(Wo[:, rows].T),
            "bq": np.ascontiguousarray(bq[rows] * s),
        }
        if not ROWGRP:
            im["mask"] = mask
        in_maps.append(im)

    global _last_in_maps
    _last_in_maps = in_maps
    res = run_bass_kernel_spmd(_NC, in_maps, list(range(N_CORES)))

    bias_term = (bo + bv @ Wo.T).astype(np.float32)   # [D]
    y = np.empty((B, T, D), dtype=np.float32)
    for b in range(B):
        y[b] = (res.results[2 * b]["y"] + res.results[2 * b + 1]["y"]
                + bias_term[None, :])
    return y
